# revision 1
# baseline (speedup 1.0000x reference)
"""Trainium2 Bass kernel for the LogicLayer (gnn_message_passing) problem.

out[n, y] = k0[y] + k1[y]*a + k2[y]*b + k3[y]*(a*b)
  with a = x[n, a_idx[y]], b = x[n, b_idx[y]],
  k = softmax(weights, -1) @ GATE_COEFFS          (per output neuron y)

Strategy (8 NeuronCores, sharded over out_dim — 2048 neurons/core, full
batch):
  * x is uploaded transposed (xT [16384, 4096], feature-major) so each
    on-device gather descriptor reads one full 16KB feature row.
  * Per-core on-device softmax of the core's weight slice gives coefficient
    tiles kg[j][q, t] = k_j(y = t*128 + q) directly (no transpose needed).
  * Per 128-output chunk t: two HW dma_gather ops (128 descriptors each,
    16KB/descriptor) land a/b rows in SBUF as A/B [128 y, 4096 n].
    ACT computes u = k1*A + k0 and v = k3*A + k2 (per-partition scale/bias),
    DVE computes v *= B, TensorE transposes u and v back to batch-major
    while accumulating u^T + v^T in PSUM, and one strided DMA per chunk
    stores the [4096, 128] output column block.
"""

import numpy as np

_GATE_COEFFS = np.array(
    [
        [0.0, 0.0, 0.0, 0.0],
        [0.0, 0.0, 0.0, 1.0],
        [0.0, 1.0, 0.0, -1.0],
        [0.0, 1.0, 0.0, 0.0],
        [0.0, 0.0, 1.0, -1.0],
        [0.0, 0.0, 1.0, 0.0],
        [0.0, 1.0, 1.0, -2.0],
        [0.0, 1.0, 1.0, -1.0],
        [1.0, -1.0, -1.0, 1.0],
        [1.0, -1.0, -1.0, 2.0],
        [1.0, 0.0, -1.0, 0.0],
        [1.0, 0.0, -1.0, 1.0],
        [1.0, -1.0, 0.0, 0.0],
        [1.0, -1.0, 0.0, 1.0],
        [1.0, 0.0, 0.0, -1.0],
        [1.0, 0.0, 0.0, 0.0],
    ],
    dtype=np.float32,
)

BATCH, IN_DIM, OUT_DIM = 4096, 16384, 16384
NCORES = 8
OC = OUT_DIM // NCORES   # 2048 outputs per core
NCHUNK = OC // 128       # 16 chunks of 128 outputs
NB = BATCH // 128        # 32 batch tiles

_PROGRAM_CACHE = {}


def _wrap_idx(idx_slice: np.ndarray) -> np.ndarray:
    """dma_gather wrapped-int16 layout per 128-index chunk: item i of chunk t
    lives at [i % 16, t*8 + i//16], replicated across the 8 16-part groups."""
    w = idx_slice.astype(np.int16).reshape(NCHUNK, 8, 16)  # [t, s, p16]
    w = np.ascontiguousarray(w.transpose(2, 0, 1)).reshape(16, NCHUNK * 8)
    return np.ascontiguousarray(np.tile(w, (8, 1)))


def _build_program():
    import concourse.bass as bass  # noqa: F401
    import concourse.tile as tile
    from concourse import bacc, mybir

    f32 = mybir.dt.float32
    i16 = mybir.dt.int16
    AF = mybir.ActivationFunctionType
    ALU = mybir.AluOpType

    nc = bacc.Bacc("TRN2", target_bir_lowering=False, debug=False)
    xT_h = nc.dram_tensor("xT", [IN_DIM, BATCH], f32, kind="ExternalInput")
    w_h = nc.dram_tensor("w16c", [OC, 16], f32, kind="ExternalInput")
    ia_h = nc.dram_tensor("ia", [128, NCHUNK * 8], i16, kind="ExternalInput")
    ib_h = nc.dram_tensor("ib", [128, NCHUNK * 8], i16, kind="ExternalInput")
    gm_h = nc.dram_tensor("gm", [4, 128, 256], f32, kind="ExternalInput")
    id_h = nc.dram_tensor("ident", [128, 128], f32, kind="ExternalInput")
    out_h = nc.dram_tensor("out", [BATCH, OC], f32, kind="ExternalOutput")

    with tile.TileContext(nc) as tc:
        from contextlib import ExitStack

        with ExitStack() as stack:
            cp = stack.enter_context(tc.tile_pool(name="const", bufs=1))

            ident = cp.tile([128, 128], f32)
            nc.sync.dma_start(ident[:], id_h.ap()[:, :])
            ia_sb = cp.tile([128, NCHUNK * 8], i16)
            nc.sync.dma_start(ia_sb[:], ia_h.ap()[:, :])
            ib_sb = cp.tile([128, NCHUNK * 8], i16)
            nc.sync.dma_start(ib_sb[:], ib_h.ap()[:, :])
            kg = [
                cp.tile([128, NCHUNK], f32, tag=f"kg{j}", name=f"kg{j}")
                for j in range(4)
            ]

            # ---- coefficients: k = softmax(weights_slice) @ GATE_COEFFS ----
            # w_sb[p, c, :] = weights row (c*128 + p); kg[j][p, c] lands in
            # exactly the per-chunk per-partition layout the ACT ops need.
            with tc.tile_pool(name="kcalc", bufs=1) as kp:
                w_sb = kp.tile([128, 256], f32, tag="wsb")
                nc.sync.dma_start(
                    w_sb[:].rearrange("p (c g) -> p c g", g=16),
                    w_h.ap().rearrange("(c p) g -> p c g", p=128),
                )
                e_sb = kp.tile([128, 256], f32, tag="esb")
                nc.scalar.activation(e_sb[:], w_sb[:], AF.Exp)
                s_sb = kp.tile([128, NCHUNK], f32, tag="ssb")
                nc.vector.tensor_reduce(
                    s_sb[:],
                    e_sb[:].rearrange("p (c g) -> p c g", g=16),
                    mybir.AxisListType.X,
                    ALU.add,
                )
                r_sb = kp.tile([128, NCHUNK], f32, tag="rsb")
                nc.vector.reciprocal(r_sb[:], s_sb[:])
                for j in range(4):
                    gm_sb = kp.tile([128, 256], f32, tag="gmsb", name=f"gm{j}")
                    nc.sync.dma_start(gm_sb[:], gm_h.ap()[j])
                    t1 = kp.tile([128, 256], f32, tag="t1", name=f"t1_{j}")
                    nc.vector.tensor_mul(t1[:], e_sb[:], gm_sb[:])
                    kraw = kp.tile([128, NCHUNK], f32, tag="kraw", name=f"kraw{j}")
                    nc.vector.tensor_reduce(
                        kraw[:],
                        t1[:].rearrange("p (c g) -> p c g", g=16),
                        mybir.AxisListType.X,
                        ALU.add,
                    )
                    nc.vector.tensor_mul(kg[j][:], kraw[:], r_sb[:])

            # ---- gather + multilinear + transpose-back + store ----
            out_ap = out_h.ap().rearrange("(a p) y -> p a y", p=128)
            with (
                tc.tile_pool(name="p2", bufs=2) as p2,
                tc.tile_pool(name="p2o", bufs=2) as p2o,
                tc.tile_pool(name="p2ps", bufs=4, space="PSUM") as p2ps,
            ):
                for t in range(NCHUNK):
                    A = p2.tile([128, 1, BATCH], f32, tag="A")
                    nc.gpsimd.dma_gather(
                        out_ap=A[:],
                        in_ap=xT_h.ap()[:, :],
                        idxs_ap=ia_sb[:, t * 8 : (t + 1) * 8],
                        num_idxs=128,
                        num_idxs_reg=128,
                        elem_size=BATCH,
                    )
                    Bt = p2.tile([128, 1, BATCH], f32, tag="B")
                    nc.gpsimd.dma_gather(
                        out_ap=Bt[:],
                        in_ap=xT_h.ap()[:, :],
                        idxs_ap=ib_sb[:, t * 8 : (t + 1) * 8],
                        num_idxs=128,
                        num_idxs_reg=128,
                        elem_size=BATCH,
                    )
                    u = p2.tile([128, BATCH], f32, tag="u")
                    v = p2.tile([128, BATCH], f32, tag="v")
                    nc.scalar.activation(
                        u[:],
                        A[:, 0, :],
                        AF.Identity,
                        bias=kg[0][:, t : t + 1],
                        scale=kg[1][:, t : t + 1],
                    )
                    nc.scalar.activation(
                        v[:],
                        A[:, 0, :],
                        AF.Identity,
                        bias=kg[2][:, t : t + 1],
                        scale=kg[3][:, t : t + 1],
                    )
                    nc.vector.tensor_mul(v[:], v[:], Bt[:, 0, :])
                    # out = transpose(u) + transpose(v*B), accumulated in PSUM
                    osb = p2o.tile([128, NB, 128], f32, tag="osb")
                    for nq in range(NB // 4):
                        ps = p2ps.tile([128, 512], f32)
                        for q in range(4):
                            nb = nq * 4 + q
                            nc.tensor.matmul(
                                ps[:, q * 128 : (q + 1) * 128],
                                u[:, nb * 128 : (nb + 1) * 128],
                                ident[:],
                                is_transpose=True,
                                start=True,
                                stop=False,
                            )
                            nc.tensor.matmul(
                                ps[:, q * 128 : (q + 1) * 128],
                                v[:, nb * 128 : (nb + 1) * 128],
                                ident[:],
                                is_transpose=True,
                                start=False,
                                stop=True,
                            )
                        nc.vector.tensor_copy(
                            osb[:, nq * 4 : (nq + 1) * 4, :].rearrange(
                                "p a y -> p (a y)"
                            ),
                            ps[:],
                        )
                    nc.sync.dma_start(
                        out_ap[:, :, t * 128 : (t + 1) * 128], osb[:]
                    )

    nc.compile()
    return nc


def _host_inputs(x, weights, a_idx, b_idx):
    x = np.asarray(x, dtype=np.float32)
    weights = np.asarray(weights, dtype=np.float32)
    a_idx = np.asarray(a_idx)
    b_idx = np.asarray(b_idx)
    xT = np.ascontiguousarray(x.T)
    gm = np.ascontiguousarray(
        np.broadcast_to(
            np.tile(_GATE_COEFFS.T, (1, 16))[:, None, :], (4, 128, 256)
        )
    ).astype(np.float32)
    ident = np.eye(128, dtype=np.float32)
    in_maps = []
    for c in range(NCORES):
        sl = slice(c * OC, (c + 1) * OC)
        in_maps.append(
            {
                "xT": xT,
                "w16c": np.ascontiguousarray(weights[sl]),
                "ia": _wrap_idx(a_idx[sl]),
                "ib": _wrap_idx(b_idx[sl]),
                "gm": gm,
                "ident": ident,
            }
        )
    return in_maps


def kernel(x, weights, a_idx, b_idx):
    from concourse.bass_utils import run_bass_kernel_spmd

    if "nc" not in _PROGRAM_CACHE:
        _PROGRAM_CACHE["nc"] = _build_program()
    nc = _PROGRAM_CACHE["nc"]

    in_maps = _host_inputs(x, weights, a_idx, b_idx)
    res = run_bass_kernel_spmd(nc, in_maps, list(range(NCORES)))
    out = np.concatenate([res.results[c]["out"] for c in range(NCORES)], axis=1)
    return out



# revision 2
# speedup vs baseline: 1.8483x; 1.8483x over previous
"""Trainium2 Bass kernel for the LogicLayer (gnn_message_passing) problem.

out[n, y] = k0[y] + k1[y]*a + k2[y]*b + k3[y]*(a*b)
  with a = x[n, a_idx[y]], b = x[n, b_idx[y]],
  k = softmax(weights, -1) @ GATE_COEFFS          (per output neuron y)

Strategy (8 NeuronCores, sharded over out_dim — 2048 neurons/core, full
batch). The kernel is HBM-bandwidth bound, so all bulk traffic is 16-bit:
  * x is uploaded transposed and recentered in fp16: xT = fp16(x - 0.5),
    [16384, 4096] feature-major. The 0.5 shift halves the fp16
    quantization error (x' in [-0.5, 0.5)) and is folded into the gate
    coefficient matrix on the host (out = C0 + C1 a' + C2 b' + C3 a'b').
  * Per-core on-device softmax of the core's weight slice gives coefficient
    tiles kg[j][q, t] = C_j(y = t*128 + q) directly.
  * Per 128-output chunk t: ONE dma_gather with 256 indices (the chunk's
    a-rows then b-rows, 8KB/descriptor, 2MB/op) lands AB [128, 2, 4096]
    in SBUF. ACT computes u = C1*A + C0 and w = C3*A + C2 (free affine,
    fp16 out), DVE computes w *= B and osb = u + w (fp16, 2x mode), and
    one contiguous DMA stores the [128, 4096] fp16 output row block.
  * Output is produced neuron-major ([out, batch] fp16); the host
    transposes/casts back to [batch, out] f32. Total HBM traffic per core:
    32MB gather + 16MB store (vs 64+32 for the f32 batch-major variant).
"""

import numpy as np

_GATE_COEFFS = np.array(
    [
        [0.0, 0.0, 0.0, 0.0],
        [0.0, 0.0, 0.0, 1.0],
        [0.0, 1.0, 0.0, -1.0],
        [0.0, 1.0, 0.0, 0.0],
        [0.0, 0.0, 1.0, -1.0],
        [0.0, 0.0, 1.0, 0.0],
        [0.0, 1.0, 1.0, -2.0],
        [0.0, 1.0, 1.0, -1.0],
        [1.0, -1.0, -1.0, 1.0],
        [1.0, -1.0, -1.0, 2.0],
        [1.0, 0.0, -1.0, 0.0],
        [1.0, 0.0, -1.0, 1.0],
        [1.0, -1.0, 0.0, 0.0],
        [1.0, -1.0, 0.0, 1.0],
        [1.0, 0.0, 0.0, -1.0],
        [1.0, 0.0, 0.0, 0.0],
    ],
    dtype=np.float32,
)

# x is stored recentered (x' = x - H); fold the shift into the coefficients:
# out = k0 + k1(a'+H) + k2(b'+H) + k3(a'+H)(b'+H) = C0 + C1 a' + C2 b' + C3 a'b'
_H = 0.5
_SHIFT_T = np.array(
    [
        [1.0, _H, _H, _H * _H],
        [0.0, 1.0, 0.0, _H],
        [0.0, 0.0, 1.0, _H],
        [0.0, 0.0, 0.0, 1.0],
    ],
    dtype=np.float32,
)
_GATE_COEFFS_C = _GATE_COEFFS @ _SHIFT_T.T  # [16, 4]: softmax(w) @ this = C

BATCH, IN_DIM, OUT_DIM = 4096, 16384, 16384
NCORES = 8
OC = OUT_DIM // NCORES   # 2048 outputs per core
NCHUNK = OC // 128       # 16 chunks of 128 outputs
IDX_PER_OP = 256         # a-rows (128) + b-rows (128) in one dma_gather

_PROGRAM_CACHE = {}


def _wrap_idx(a_slice: np.ndarray, b_slice: np.ndarray) -> np.ndarray:
    """dma_gather wrapped-int16 layout. Per chunk t the op gathers 256 rows
    (chunk's 128 a-idxs then 128 b-idxs); item i of op t lives at
    [i % 16, t*16 + i//16], replicated across the 8 16-partition groups."""
    idx = np.stack([a_slice.reshape(NCHUNK, 128), b_slice.reshape(NCHUNK, 128)], 1)
    idx = idx.reshape(NCHUNK, IDX_PER_OP).astype(np.int16)      # [t, i]
    w = idx.reshape(NCHUNK, IDX_PER_OP // 16, 16)               # [t, s, p16]
    w = np.ascontiguousarray(w.transpose(2, 0, 1)).reshape(16, NCHUNK * 16)
    return np.ascontiguousarray(np.tile(w, (8, 1)))


def _build_program():
    import concourse.bass as bass  # noqa: F401
    import concourse.tile as tile
    from concourse import bacc, mybir

    f32 = mybir.dt.float32
    f16 = mybir.dt.float16
    i16 = mybir.dt.int16
    AF = mybir.ActivationFunctionType
    ALU = mybir.AluOpType

    nc = bacc.Bacc("TRN2", target_bir_lowering=False, debug=False)
    xT_h = nc.dram_tensor("xT", [IN_DIM, BATCH], f16, kind="ExternalInput")
    w_h = nc.dram_tensor("w16c", [OC, 16], f32, kind="ExternalInput")
    iab_h = nc.dram_tensor("iab", [128, NCHUNK * 16], i16, kind="ExternalInput")
    gm_h = nc.dram_tensor("gm", [4, 128, 256], f32, kind="ExternalInput")
    out_h = nc.dram_tensor("out", [OC, BATCH], f16, kind="ExternalOutput")

    with tile.TileContext(nc) as tc:
        from contextlib import ExitStack

        with ExitStack() as stack:
            cp = stack.enter_context(tc.tile_pool(name="const", bufs=1))

            iab_sb = cp.tile([128, NCHUNK * 16], i16)
            nc.sync.dma_start(iab_sb[:], iab_h.ap()[:, :])
            kg = [
                cp.tile([128, NCHUNK], f32, tag=f"kg{j}", name=f"kg{j}")
                for j in range(4)
            ]

            # ---- coefficients: C = softmax(weights_slice) @ GATE_COEFFS_C ----
            # w_sb[p, c, :] = weights row (c*128 + p); kg[j][p, c] lands in
            # exactly the per-chunk per-partition layout the ACT ops need.
            with tc.tile_pool(name="kcalc", bufs=1) as kp:
                w_sb = kp.tile([128, 256], f32, tag="wsb")
                nc.sync.dma_start(
                    w_sb[:].rearrange("p (c g) -> p c g", g=16),
                    w_h.ap().rearrange("(c p) g -> p c g", p=128),
                )
                e_sb = kp.tile([128, 256], f32, tag="esb")
                nc.scalar.activation(e_sb[:], w_sb[:], AF.Exp)
                s_sb = kp.tile([128, NCHUNK], f32, tag="ssb")
                nc.vector.tensor_reduce(
                    s_sb[:],
                    e_sb[:].rearrange("p (c g) -> p c g", g=16),
                    mybir.AxisListType.X,
                    ALU.add,
                )
                r_sb = kp.tile([128, NCHUNK], f32, tag="rsb")
                nc.vector.reciprocal(r_sb[:], s_sb[:])
                for j in range(4):
                    gm_sb = kp.tile([128, 256], f32, tag="gmsb", name=f"gm{j}")
                    nc.sync.dma_start(gm_sb[:], gm_h.ap()[j])
                    t1 = kp.tile([128, 256], f32, tag="t1", name=f"t1_{j}")
                    nc.vector.tensor_mul(t1[:], e_sb[:], gm_sb[:])
                    kraw = kp.tile([128, NCHUNK], f32, tag="kraw", name=f"kraw{j}")
                    nc.vector.tensor_reduce(
                        kraw[:],
                        t1[:].rearrange("p (c g) -> p c g", g=16),
                        mybir.AxisListType.X,
                        ALU.add,
                    )
                    nc.vector.tensor_mul(kg[j][:], kraw[:], r_sb[:])

            # ---- gather + multilinear + store (all fp16, neuron-major) ----
            with (
                tc.tile_pool(name="pab", bufs=3) as pab,
                tc.tile_pool(name="puw", bufs=2) as puw,
                tc.tile_pool(name="po", bufs=2) as po,
            ):
                for t in range(NCHUNK):
                    AB = pab.tile([128, 2, BATCH], f16, tag="AB")
                    nc.gpsimd.dma_gather(
                        out_ap=AB[:],
                        in_ap=xT_h.ap()[:, :],
                        idxs_ap=iab_sb[:, t * 16 : (t + 1) * 16],
                        num_idxs=IDX_PER_OP,
                        num_idxs_reg=IDX_PER_OP,
                        elem_size=BATCH,
                    )
                    u = puw.tile([128, BATCH], f16, tag="u")
                    w = puw.tile([128, BATCH], f16, tag="w")
                    nc.scalar.activation(
                        u[:],
                        AB[:, 0, :],
                        AF.Identity,
                        bias=kg[0][:, t : t + 1],
                        scale=kg[1][:, t : t + 1],
                    )
                    nc.scalar.activation(
                        w[:],
                        AB[:, 0, :],
                        AF.Identity,
                        bias=kg[2][:, t : t + 1],
                        scale=kg[3][:, t : t + 1],
                    )
                    nc.vector.tensor_mul(w[:], w[:], AB[:, 1, :])
                    osb = po.tile([128, BATCH], f16, tag="osb")
                    nc.vector.tensor_add(osb[:], u[:], w[:])
                    nc.sync.dma_start(
                        out_h.ap()[t * 128 : (t + 1) * 128, :], osb[:]
                    )

    nc.compile()
    return nc


def _host_inputs(x, weights, a_idx, b_idx):
    x = np.asarray(x, dtype=np.float32)
    weights = np.asarray(weights, dtype=np.float32)
    a_idx = np.asarray(a_idx)
    b_idx = np.asarray(b_idx)
    xT = np.ascontiguousarray((x.T - np.float32(_H)).astype(np.float16))
    gm = np.ascontiguousarray(
        np.broadcast_to(
            np.tile(_GATE_COEFFS_C.T, (1, 16))[:, None, :], (4, 128, 256)
        )
    ).astype(np.float32)
    in_maps = []
    for c in range(NCORES):
        sl = slice(c * OC, (c + 1) * OC)
        in_maps.append(
            {
                "xT": xT,
                "w16c": np.ascontiguousarray(weights[sl]),
                "iab": _wrap_idx(a_idx[sl], b_idx[sl]),
                "gm": gm,
            }
        )
    return in_maps


def kernel(x, weights, a_idx, b_idx):
    from concourse.bass_utils import run_bass_kernel_spmd

    if "nc" not in _PROGRAM_CACHE:
        _PROGRAM_CACHE["nc"] = _build_program()
    nc = _PROGRAM_CACHE["nc"]

    in_maps = _host_inputs(x, weights, a_idx, b_idx)
    res = run_bass_kernel_spmd(nc, in_maps, list(range(NCORES)))
    outT = np.concatenate(
        [np.asarray(res.results[c]["out"]) for c in range(NCORES)], axis=0
    )
    return outT.T.astype(np.float32)


# revision 4
# speedup vs baseline: 1.8531x; 1.0026x over previous
"""Trainium2 Bass kernel for the LogicLayer (gnn_message_passing) problem.

out[n, y] = k0[y] + k1[y]*a + k2[y]*b + k3[y]*(a*b)
  with a = x[n, a_idx[y]], b = x[n, b_idx[y]],
  k = softmax(weights, -1) @ GATE_COEFFS          (per output neuron y)

Strategy (8 NeuronCores, sharded over out_dim — 2048 neurons/core, full
batch). The kernel is HBM-bandwidth bound, so all bulk traffic is 16-bit:
  * x is uploaded transposed and recentered in fp16: xT = fp16(x - 0.5),
    [16384, 4096] feature-major. The 0.5 shift halves the fp16
    quantization error (x' in [-0.5, 0.5)) and is folded into the gate
    coefficient matrix on the host (out = C0 + C1 a' + C2 b' + C3 a'b').
  * Per-core on-device softmax of the core's weight slice gives coefficient
    tiles kg[j][q, t] = C_j(y = t*128 + q) directly.
  * Per 128-output chunk t: ONE dma_gather with 256 indices (the chunk's
    a-rows then b-rows, 8KB/descriptor, 2MB/op) lands AB [128, 2, 4096]
    in SBUF. ACT computes u = C1*A + C0 and w = C3*A + C2 (free affine,
    fp16 out), DVE computes w *= B and osb = u + w (fp16, 2x mode), and
    one contiguous DMA stores the [128, 4096] fp16 output row block.
  * Output is produced neuron-major ([out, batch] fp16); the host
    transposes/casts back to [batch, out] f32. Total HBM traffic per core:
    32MB gather + 16MB store (vs 64+32 for the f32 batch-major variant).
"""

import numpy as np

_GATE_COEFFS = np.array(
    [
        [0.0, 0.0, 0.0, 0.0],
        [0.0, 0.0, 0.0, 1.0],
        [0.0, 1.0, 0.0, -1.0],
        [0.0, 1.0, 0.0, 0.0],
        [0.0, 0.0, 1.0, -1.0],
        [0.0, 0.0, 1.0, 0.0],
        [0.0, 1.0, 1.0, -2.0],
        [0.0, 1.0, 1.0, -1.0],
        [1.0, -1.0, -1.0, 1.0],
        [1.0, -1.0, -1.0, 2.0],
        [1.0, 0.0, -1.0, 0.0],
        [1.0, 0.0, -1.0, 1.0],
        [1.0, -1.0, 0.0, 0.0],
        [1.0, -1.0, 0.0, 1.0],
        [1.0, 0.0, 0.0, -1.0],
        [1.0, 0.0, 0.0, 0.0],
    ],
    dtype=np.float32,
)

# x is stored recentered (x' = x - H); fold the shift into the coefficients:
# out = k0 + k1(a'+H) + k2(b'+H) + k3(a'+H)(b'+H) = C0 + C1 a' + C2 b' + C3 a'b'
_H = 0.5
_SHIFT_T = np.array(
    [
        [1.0, _H, _H, _H * _H],
        [0.0, 1.0, 0.0, _H],
        [0.0, 0.0, 1.0, _H],
        [0.0, 0.0, 0.0, 1.0],
    ],
    dtype=np.float32,
)
_GATE_COEFFS_C = _GATE_COEFFS @ _SHIFT_T.T  # [16, 4]: softmax(w) @ this = C

BATCH, IN_DIM, OUT_DIM = 4096, 16384, 16384
NCORES = 8
OC = OUT_DIM // NCORES   # 2048 outputs per core
NCHUNK = OC // 128       # 16 chunks of 128 outputs
IDX_PER_OP = 256         # a-rows (128) + b-rows (128) in one dma_gather

_PROGRAM_CACHE = {}


def _wrap_idx(a_slice: np.ndarray, b_slice: np.ndarray) -> np.ndarray:
    """dma_gather wrapped-int16 layout. Per chunk t the op gathers 256 rows
    (chunk's 128 a-idxs then 128 b-idxs); item i of op t lives at
    [i % 16, t*16 + i//16], replicated across the 8 16-partition groups."""
    idx = np.stack([a_slice.reshape(NCHUNK, 128), b_slice.reshape(NCHUNK, 128)], 1)
    idx = idx.reshape(NCHUNK, IDX_PER_OP).astype(np.int16)      # [t, i]
    w = idx.reshape(NCHUNK, IDX_PER_OP // 16, 16)               # [t, s, p16]
    w = np.ascontiguousarray(w.transpose(2, 0, 1)).reshape(16, NCHUNK * 16)
    return np.ascontiguousarray(np.tile(w, (8, 1)))


def _build_program():
    import concourse.bass as bass  # noqa: F401
    import concourse.tile as tile
    from concourse import bacc, mybir

    f32 = mybir.dt.float32
    f16 = mybir.dt.float16
    i16 = mybir.dt.int16
    AF = mybir.ActivationFunctionType
    ALU = mybir.AluOpType

    nc = bacc.Bacc("TRN2", target_bir_lowering=False, debug=False)
    xT_h = nc.dram_tensor("xT", [IN_DIM, BATCH], f16, kind="ExternalInput")
    w_h = nc.dram_tensor("w16c", [OC, 16], f32, kind="ExternalInput")
    iab_h = nc.dram_tensor("iab", [128, NCHUNK * 16], i16, kind="ExternalInput")
    gm_h = nc.dram_tensor("gm", [4, 128, 256], f32, kind="ExternalInput")
    out_h = nc.dram_tensor("out", [OC, BATCH], f16, kind="ExternalOutput")

    with tile.TileContext(nc) as tc:
        from contextlib import ExitStack

        with ExitStack() as stack:
            cp = stack.enter_context(tc.tile_pool(name="const", bufs=1))
            # all pools coexist: no SBUF reuse between the coefficient calc
            # and the main loop, so the first gathers start immediately and
            # overlap the (serial, small) softmax chain.
            kp = stack.enter_context(tc.tile_pool(name="kcalc", bufs=1))
            pab = stack.enter_context(tc.tile_pool(name="pab", bufs=6))
            puw = stack.enter_context(tc.tile_pool(name="puw", bufs=2))
            po = stack.enter_context(tc.tile_pool(name="po", bufs=2))

            iab_sb = cp.tile([128, NCHUNK * 16], i16)
            nc.sync.dma_start(iab_sb[:], iab_h.ap()[:, :])
            kg = [
                cp.tile([128, NCHUNK], f32, tag=f"kg{j}", name=f"kg{j}")
                for j in range(4)
            ]

            # ---- coefficients: C = softmax(weights_slice) @ GATE_COEFFS_C ----
            # w_sb[p, c, :] = weights row (c*128 + p); kg[j][p, c] lands in
            # exactly the per-chunk per-partition layout the ACT ops need.
            w_sb = kp.tile([128, 256], f32, tag="wsb")
            nc.sync.dma_start(
                w_sb[:].rearrange("p (c g) -> p c g", g=16),
                w_h.ap().rearrange("(c p) g -> p c g", p=128),
            )
            e_sb = kp.tile([128, 256], f32, tag="esb")
            nc.scalar.activation(e_sb[:], w_sb[:], AF.Exp)
            s_sb = kp.tile([128, NCHUNK], f32, tag="ssb")
            nc.vector.tensor_reduce(
                s_sb[:],
                e_sb[:].rearrange("p (c g) -> p c g", g=16),
                mybir.AxisListType.X,
                ALU.add,
            )
            r_sb = kp.tile([128, NCHUNK], f32, tag="rsb")
            nc.vector.reciprocal(r_sb[:], s_sb[:])
            for j in range(4):
                gm_sb = kp.tile([128, 256], f32, tag="gmsb", name=f"gm{j}")
                nc.sync.dma_start(gm_sb[:], gm_h.ap()[j])
                t1 = kp.tile([128, 256], f32, tag="t1", name=f"t1_{j}")
                nc.vector.tensor_mul(t1[:], e_sb[:], gm_sb[:])
                kraw = kp.tile([128, NCHUNK], f32, tag="kraw", name=f"kraw{j}")
                nc.vector.tensor_reduce(
                    kraw[:],
                    t1[:].rearrange("p (c g) -> p c g", g=16),
                    mybir.AxisListType.X,
                    ALU.add,
                )
                nc.vector.tensor_mul(kg[j][:], kraw[:], r_sb[:])

            # ---- gather + multilinear + store (all fp16, neuron-major) ----
            for t in range(NCHUNK):
                AB = pab.tile([128, 2, BATCH], f16, tag="AB")
                nc.gpsimd.dma_gather(
                    out_ap=AB[:],
                    in_ap=xT_h.ap()[:, :],
                    idxs_ap=iab_sb[:, t * 16 : (t + 1) * 16],
                    num_idxs=IDX_PER_OP,
                    num_idxs_reg=IDX_PER_OP,
                    elem_size=BATCH,
                )
                u = puw.tile([128, BATCH], f16, tag="u")
                w = puw.tile([128, BATCH], f16, tag="w")
                nc.scalar.activation(
                    u[:],
                    AB[:, 0, :],
                    AF.Identity,
                    bias=kg[0][:, t : t + 1],
                    scale=kg[1][:, t : t + 1],
                )
                nc.scalar.activation(
                    w[:],
                    AB[:, 0, :],
                    AF.Identity,
                    bias=kg[2][:, t : t + 1],
                    scale=kg[3][:, t : t + 1],
                )
                nc.vector.tensor_mul(w[:], w[:], AB[:, 1, :])
                if t % 2 == 0:
                    osb = po.tile([128, 2, BATCH], f16, tag="osb")
                nc.vector.tensor_add(osb[:, t % 2, :], u[:], w[:])
                if t % 2 == 1:
                    # store two chunks per DMA op (2MB) to amortize per-op cost
                    nc.sync.dma_start(
                        out_h.ap()[(t - 1) * 128 : (t + 1) * 128, :].rearrange(
                            "(c p) n -> p c n", p=128
                        ),
                        osb[:],
                    )

    nc.compile()
    return nc


def _host_inputs(x, weights, a_idx, b_idx):
    x = np.asarray(x, dtype=np.float32)
    weights = np.asarray(weights, dtype=np.float32)
    a_idx = np.asarray(a_idx)
    b_idx = np.asarray(b_idx)
    xT = np.ascontiguousarray((x.T - np.float32(_H)).astype(np.float16))
    gm = np.ascontiguousarray(
        np.broadcast_to(
            np.tile(_GATE_COEFFS_C.T, (1, 16))[:, None, :], (4, 128, 256)
        )
    ).astype(np.float32)
    in_maps = []
    for c in range(NCORES):
        sl = slice(c * OC, (c + 1) * OC)
        in_maps.append(
            {
                "xT": xT,
                "w16c": np.ascontiguousarray(weights[sl]),
                "iab": _wrap_idx(a_idx[sl], b_idx[sl]),
                "gm": gm,
            }
        )
    return in_maps


def kernel(x, weights, a_idx, b_idx):
    from concourse.bass_utils import run_bass_kernel_spmd

    if "nc" not in _PROGRAM_CACHE:
        _PROGRAM_CACHE["nc"] = _build_program()
    nc = _PROGRAM_CACHE["nc"]

    in_maps = _host_inputs(x, weights, a_idx, b_idx)
    res = run_bass_kernel_spmd(nc, in_maps, list(range(NCORES)))
    outT = np.concatenate(
        [np.asarray(res.results[c]["out"]) for c in range(NCORES)], axis=0
    )
    return outT.T.astype(np.float32)
